# revision 1
# baseline (speedup 1.0000x reference)
"""Multi-head "genetic" attention (windowed-causal, GQA) for Trainium2.

Self-contained: kernel(**inputs) takes full inputs, shards across 8
NeuronCores (2 query heads per core; value head h//4 per GQA), runs a
Bass/Tile kernel per core, and reduces the row-sharded output projection
partials on host.

Precision strategy: the score/softmax-weight path is scale-insensitive
(fitness multiplies tiny logits), so q/k/scores run in bf16; the value
path (v, attention-weighted v, output projection) runs in fp32r
(fp32 storage, ~1e-4 matmul rounding) since its error reaches the
output linearly.

Shapes (hardcoded): x (1, 2048, 1024), H=16 heads, head_dim 64, HV=4
value heads, window 512 (causal band of 513).
"""

import numpy as np

import bass_rust
import concourse.bass as bass
import concourse.tile as tile
from concourse import mybir
from concourse.bass_utils import run_bass_kernel_spmd
from concourse.masks import make_identity

F32 = mybir.dt.float32
F32R = mybir.dt.float32r
BF16 = mybir.dt.bfloat16
AF = mybir.ActivationFunctionType
ALU = mybir.AluOpType

T, D, H, HD, HV, WIN = 2048, 1024, 16, 64, 4, 512
NCORES = 8
HPC = H // NCORES          # 2 heads per core
P = 128
TT = T // P                # 16 t-tiles
KT = D // P                # 8 k-tiles over d_model
QKW = HPC * HD             # 128 q (or k) columns per core
VW = HD                    # 64 v columns per core
QKVW = 2 * QKW + VW        # 320 fused projection columns
EPS = 1.1920929e-07
NB = WIN // P + 1          # 5 band s-tiles max
MASK_FILL = -1.0e6         # exp(fill * fitness) == 0 for any fitness here

# ---------------------------------------------------------------------------
# This walrus build rejects >1 sem wait per instruction ("Too many sync wait
# commands"). Move extra waits onto same-engine NOPs inserted just before the
# offending instruction (engine queues are in-order, so blocking on the NOP
# is equivalent to blocking on the instruction itself).
_MAX_WAITS = 1


def split_multi_waits(nc, max_waits=_MAX_WAITS):
    for bb in nc.main_func.blocks:
        insts = bb.instructions
        i = 0
        while i < len(insts):
            inst = insts[i]
            si = inst.sync_info
            waits = list(si.on_wait or []) if si is not None else []
            if len(waits) > max_waits:
                si.on_wait = waits[-max_waits:]
                extra = waits[:-max_waits]
                nops = []
                for j in range(0, len(extra), max_waits):
                    n = nc.engines[inst.engine].nop(nofuse=True)
                    ni = n.ins
                    for bb2 in nc.main_func.blocks:
                        if ni in bb2.instructions:
                            bb2.instructions.remove(ni)
                            break
                    chunk = extra[j : j + max_waits]
                    if ni.sync_info is None:
                        ni.sync_info = bass_rust.SyncInfo(on_wait=chunk, on_update=[])
                    else:
                        ni.sync_info.on_wait = chunk
                    nops.append(ni)
                for k, ni in enumerate(nops):
                    insts.insert(i + k, ni)
                i += len(nops)
            i += 1
# ---------------------------------------------------------------------------


def _broadcast_row_ap(dram_ap, width):
    """DRAM AP replicating a (1, width) row across all 128 partitions."""
    return bass.AP(
        tensor=dram_ap.tensor,
        offset=dram_ap.offset,
        ap=[[0, P], [1, width]],
    )


def build_kernel(nc, tc, xT_d, wqkv_d, bqkv_d, rmsw_d, wo_d, out_d):
    from contextlib import ExitStack

    with ExitStack() as ctx:
        consts = ctx.enter_context(tc.tile_pool(name="consts", bufs=1))
        persist = ctx.enter_context(tc.tile_pool(name="persist", bufs=1))

        ident_bf = consts.tile([P, P], BF16)
        make_identity(nc, ident_bf)
        ident_f = consts.tile([P, P], F32)
        make_identity(nc, ident_f)

        eps_t = consts.tile([P, 1], F32)
        nc.vector.memset(eps_t, EPS)
        ones_f = consts.tile([P, 1], F32)
        nc.vector.memset(ones_f, 1.0)

        # 127 - p, used for the partial-band diagonal t-tiles (s_lo == 0)
        causal_cnt = consts.tile([P, P], F32)
        nc.gpsimd.memset(causal_cnt, 1.0)
        nc.gpsimd.affine_select(
            out=causal_cnt, in_=causal_cnt, compare_op=ALU.is_ge, fill=0.0,
            base=0, pattern=[[-1, P]], channel_multiplier=1,
        )
        corr_lt = consts.tile([P, 1], F32)
        nc.vector.reduce_sum(corr_lt, causal_cnt, axis=mybir.AxisListType.X)
        nc.vector.tensor_scalar(corr_lt, corr_lt, -1.0, 128.0, ALU.mult, ALU.add)

        # denominator offset per t-tile: rs/T + Cvec ; in-strip masked slots
        # produce sigmoid(-1e6)=0, so their 0.5 contribution moves here.
        cvec = {}
        for tt in range(min(NB - 1, TT)):
            W = (tt + 1) * P
            c_base = 0.5 * (T - W) / T + 0.5
            cv = consts.tile([P, 1], F32, tag=f"cvec{tt}")
            nc.vector.tensor_scalar(cv, corr_lt, 0.5 / T, c_base, ALU.mult, ALU.add)
            cvec[tt] = cv
        C_FULL = 0.5 * (T - NB * P + (P - 1)) / T + 0.5

        fill_mask = nc.gpsimd.to_reg(MASK_FILL)
        fill_zero = nc.gpsimd.to_reg(0.0)

        wqkv_sb = persist.tile([P, KT, QKVW], F32R)
        nc.sync.dma_start(wqkv_sb, wqkv_d.rearrange("(ko p) n -> p ko n", p=P))
        wo_sb = persist.tile([P, D], F32R)
        nc.sync.dma_start(wo_sb, wo_d[:])

        bias_b = consts.tile([P, QKVW], F32)
        nc.gpsimd.dma_start(bias_b, _broadcast_row_ap(bqkv_d[:], QKVW))
        rmsw_b = consts.tile([P, 2 * QKW], F32)
        nc.gpsimd.dma_start(rmsw_b, _broadcast_row_ap(rmsw_d[:], 2 * QKW))

        ones2 = consts.tile([2, P], F32)
        nc.vector.memset(ones2, 1.0)

        qT = persist.tile([P, T], BF16)     # rows: head0 dims 0-63, head1 64-127
        kT = persist.tile([P, T], BF16)
        vN = persist.tile([P, TT, VW + 2], F32R)  # v natural + ones col (row sums) + pad
        recip_all = persist.tile([P, HPC, TT], F32)
        nc.vector.tensor_copy(
            vN[:, :, VW : VW + 2],
            ones_f[:, :, None].to_broadcast((P, TT, 2)),
        )
        fs_all = persist.tile([P, HPC, TT], F32)

        xT_t = xT_d.rearrange("(ko p) t -> p ko t", p=P)

        # ---------------- Phase A: QKV projection + RMSNorm + transposes
        with tc.tile_pool(name="ab_sb", bufs=3) as ab_sb, \
             tc.tile_pool(name="ab_ps", bufs=2, space="PSUM") as ab_ps, \
             tc.tile_pool(name="ab_tr", bufs=2, space="PSUM") as ab_tr:
            for tt in range(TT):
                xTs = ab_sb.tile([P, KT, P], F32R, tag="xT")
                nc.gpsimd.dma_start(xTs, xT_t[:, :, tt * P : (tt + 1) * P])

                qkv_ps = ab_ps.tile([P, QKVW], F32, tag="qkv")
                for ko in range(KT):
                    nc.tensor.matmul(
                        qkv_ps, lhsT=xTs[:, ko, :], rhs=wqkv_sb[:, ko, :],
                        start=(ko == 0), stop=(ko == KT - 1),
                    )
                qkv_sb = ab_sb.tile([P, QKVW], F32, tag="qkv_sb")
                nc.vector.tensor_add(qkv_sb, qkv_ps, bias_b)

                # RMSNorm over each 64-wide head chunk of q and k
                sq = ab_sb.tile([P, 2 * QKW], F32, tag="sq")
                nc.vector.tensor_mul(sq, qkv_sb[:, : 2 * QKW], qkv_sb[:, : 2 * QKW])
                ssum = ab_sb.tile([P, 4], F32, tag="ssum")
                nc.vector.reduce_sum(
                    ssum, sq.rearrange("p (c d) -> p c d", d=HD),
                    axis=mybir.AxisListType.X,
                )
                fac = ab_sb.tile([P, 4], F32, tag="fac")
                nc.scalar.activation(fac, ssum, AF.Sqrt, bias=eps_t, scale=1.0 / HD)
                rfac = ab_sb.tile([P, 4], F32, tag="rfac")
                nc.vector.reciprocal(rfac, fac)
                qkn = ab_sb.tile([P, 4, HD], BF16, tag="qkn")
                qk = qkv_sb[:, : 2 * QKW].rearrange("p (c d) -> p c d", d=HD)
                nc.vector.tensor_tensor(
                    qkn, qk, rfac[:, :, None].to_broadcast((P, 4, HD)), ALU.mult
                )
                # rms weight (q halves pre-scaled by 1/8 on host)
                nc.vector.tensor_tensor(
                    qkn, qkn,
                    rmsw_b.rearrange("p (c d) -> p c d", d=HD), ALU.mult,
                )

                for j, dst in ((0, qT), (1, kT)):
                    ps = ab_tr.tile([P, P], BF16, tag="tr")
                    nc.tensor.transpose(
                        ps, qkn[:, 2 * j : 2 * j + 2, :].rearrange("p c d -> p (c d)"),
                        ident_bf,
                    )
                    if j == 0:
                        nc.vector.tensor_copy(dst[:, tt * P : (tt + 1) * P], ps)
                    else:
                        nc.scalar.copy(dst[:, tt * P : (tt + 1) * P], ps)
                nc.gpsimd.tensor_copy(vN[:, tt, :VW], qkv_sb[:, 2 * QKW :])

        # ---------------- Pass 1: banded scores (bf16), sigmoid stats
        strips = {}
        strip_pool = ctx.enter_context(tc.tile_pool(name="strips", bufs=1))
        with tc.tile_pool(name="p1_sb", bufs=3) as p1_sb, \
             tc.tile_pool(name="p1_ps", bufs=3, space="PSUM") as p1_ps:
            for tt in range(TT):
                s_lo = max(0, tt - (NB - 1))
                nst = tt - s_lo + 1
                W = nst * P
                for h in range(HPC):
                    ps = p1_ps.tile([P, NB * P], F32, tag="S")
                    for c0 in range(0, W, 512):
                        cw = min(512, W - c0)
                        nc.tensor.matmul(
                            ps[:, c0 : c0 + cw],
                            lhsT=qT[h * HD : (h + 1) * HD, tt * P : (tt + 1) * P],
                            rhs=kT[h * HD : (h + 1) * HD,
                                   s_lo * P + c0 : s_lo * P + c0 + cw],
                            start=True, stop=True,
                        )
                    strip = strip_pool.tile([P, W], BF16, tag=f"st{h}_{tt}")
                    strips[(h, tt)] = strip
                    if h == 0:
                        nc.vector.tensor_copy(strip, ps[:, :W])
                    else:
                        nc.scalar.copy(strip, ps[:, :W])
                    # band masking: keep c <= p on the diagonal tile,
                    # c >= p on the leading tile of full strips
                    nc.gpsimd.affine_select(
                        out=strip[:, W - P : W], in_=strip[:, W - P : W],
                        compare_op=ALU.is_ge, fill=fill_mask,
                        base=0, pattern=[[-1, P]], channel_multiplier=1,
                    )
                    if nst == NB:
                        nc.gpsimd.affine_select(
                            out=strip[:, :P], in_=strip[:, :P],
                            compare_op=ALU.is_ge, fill=fill_mask,
                            base=0, pattern=[[1, P]], channel_multiplier=-1,
                        )

                    sig = p1_sb.tile([P, NB * P], F32, tag="sig")
                    rs = p1_sb.tile([P, 1], F32, tag="rs")
                    nc.scalar.activation(sig[:, :W], strip, AF.Sigmoid, accum_out=rs)
                    den = p1_sb.tile([P, 1], F32, tag="den")
                    if nst == NB:
                        nc.vector.tensor_scalar(den, rs, 1.0 / T, C_FULL,
                                                ALU.mult, ALU.add)
                    else:
                        nc.vector.tensor_scalar(den, rs, 1.0 / T, cvec[tt],
                                                ALU.mult, ALU.add)
                    nc.vector.reciprocal(recip_all[:, h, tt : tt + 1], den)

            # gene fitness scale per (head, t): recip(t) / sum_t recip(t).
            # Cross-partition sum via PE ones-reduction, then a DRAM bounce
            # to broadcast the two per-head scalars back across partitions.
            rsum = p1_sb.tile([P, HPC], F32, tag="rsum")
            nc.vector.reduce_sum(rsum, recip_all, axis=mybir.AxisListType.X)
            with tc.tile_pool(name="p1_sp", bufs=1, space="PSUM") as p1_sp:
                sinv_ps = p1_sp.tile([HPC, 1], F32, tag="sp")
                nc.tensor.matmul(sinv_ps, lhsT=rsum, rhs=ones_f,
                                 start=True, stop=True)
                sinv_r = p1_sb.tile([HPC, 1], F32, tag="sinvr")
                nc.vector.reciprocal(sinv_r, sinv_ps)
                # broadcast the two per-head scalars across partitions on-chip:
                # ones2.T @ diag(sinv_r) puts [s0, s1] on every partition
                diag2 = p1_sb.tile([HPC, HPC], F32, tag="diag2")
                nc.vector.tensor_copy(diag2, sinv_r.to_broadcast((HPC, HPC)))
                nc.gpsimd.affine_select(
                    out=diag2, in_=diag2, compare_op=ALU.is_equal, fill=fill_zero,
                    base=0, pattern=[[-1, HPC]], channel_multiplier=1,
                )
                srb_ps = p1_sp.tile([P, HPC], F32, tag="srbp")
                nc.tensor.matmul(srb_ps, lhsT=ones2, rhs=diag2,
                                 start=True, stop=True)
                srb = p1_sb.tile([P, HPC], F32, tag="srb")
                nc.vector.tensor_copy(srb, srb_ps)
            nc.vector.tensor_tensor(
                fs_all, recip_all,
                srb[:, :, None].to_broadcast((P, HPC, TT)), ALU.mult,
            )

        # all fitness diagonals up front, off pass 2's critical path
        dmat_pool = ctx.enter_context(tc.tile_pool(name="dmats", bufs=1))
        dmats = {}
        for tt in range(TT):
            for h in range(HPC):
                dm = dmat_pool.tile([P, P], BF16, tag=f"dm{h}_{tt}")
                nc.gpsimd.tensor_copy(
                    dm, fs_all[:, h, tt : tt + 1].to_broadcast((P, P))
                )
                nc.gpsimd.affine_select(
                    out=dm, in_=dm, compare_op=ALU.is_equal, fill=fill_zero,
                    base=0, pattern=[[-1, P]], channel_multiplier=1,
                )
                dmats[(h, tt)] = dm

        # ---------------- Pass 2: fitness-scaled eT via diag-matmul, AV,
        # output projection
        with tc.tile_pool(name="p2_sb", bufs=3) as p2_sb, \
             tc.tile_pool(name="p2_wt", bufs=2, space="PSUM") as p2_wt, \
             tc.tile_pool(name="p2_av", bufs=2, space="PSUM") as p2_av, \
             tc.tile_pool(name="p2_at", bufs=1, space="PSUM") as p2_at, \
             tc.tile_pool(name="p2_o", bufs=1, space="PSUM") as p2_o:
            for tt in range(TT):
                s_lo = max(0, tt - (NB - 1))
                nst = tt - s_lo + 1
                W = nst * P
                attn = p2_sb.tile([P, QKW], F32, tag="attn")
                for h in range(HPC):
                    dmat = dmats[(h, tt)]
                    av_ps = p2_av.tile([P, VW + 2], F32, tag="av")
                    eTs = []
                    st = 0
                    while st < nst:
                        pw = min(2, nst - st)  # pair transposes per exp
                        wt_ps = p2_wt.tile([P, 2, P], F32, tag="wt")
                        for k in range(pw):
                            # wT(s,t) = strip(t,s) * fitness(t) via lhsT.T @ diag
                            nc.tensor.matmul(
                                wt_ps[:, k, :],
                                lhsT=strips[(h, tt)][:, (st + k) * P : (st + k + 1) * P],
                                rhs=dmat, start=True, stop=True,
                            )
                        eT = p2_sb.tile([P, 2, P], F32R, tag="eT", bufs=5)
                        nc.scalar.activation(
                            eT[:, :pw, :], wt_ps[:, :pw, :], AF.Exp
                        )
                        eTs.append((st, pw, eT))
                        st += pw
                    for st, pw, eT in eTs:
                        for k in range(pw):
                            nc.tensor.matmul(
                                av_ps, lhsT=eT[:, k, :],
                                rhs=vN[:, s_lo + st + k, :],
                                start=(st + k == 0), stop=(st + k == nst - 1),
                            )
                    erec = p2_sb.tile([P, 1], F32, tag="erec")
                    nc.vector.reciprocal(erec, av_ps[:, VW : VW + 1])
                    nc.vector.tensor_tensor(
                        attn[:, h * VW : (h + 1) * VW], av_ps[:, :VW],
                        erec.to_broadcast((P, VW)), ALU.mult,
                    )

                atp = p2_at.tile([P, P], F32, tag="atp")
                nc.tensor.transpose(atp, attn, ident_f)
                atT = p2_sb.tile([P, P], F32R, tag="atT")
                nc.scalar.copy(atT, atp)
                out_sb = p2_sb.tile([P, D], F32, tag="osb")
                for ci, c0 in enumerate(range(0, D, 512)):
                    ops = p2_o.tile([P, 512], F32, tag="o")
                    nc.tensor.matmul(
                        ops, lhsT=atT, rhs=wo_sb[:, c0 : c0 + 512],
                        start=True, stop=True,
                    )
                    if ci == 0:
                        nc.vector.tensor_copy(out_sb[:, c0 : c0 + 512], ops)
                    else:
                        nc.scalar.copy(out_sb[:, c0 : c0 + 512], ops)
                nc.sync.dma_start(out_d[tt * P : (tt + 1) * P, :], out_sb)


def build_nc(repeats=1):
    nc = bass.Bass()
    xT_d = nc.declare_dram_parameter("xT", [D, T], F32R, isOutput=False)
    wqkv_d = nc.declare_dram_parameter("wqkv", [D, QKVW], F32R, isOutput=False)
    bqkv_d = nc.declare_dram_parameter("bqkv", [1, QKVW], F32, isOutput=False)
    rmsw_d = nc.declare_dram_parameter("rmsw", [1, 2 * QKW], F32, isOutput=False)
    wo_d = nc.declare_dram_parameter("wo", [QKW, D], F32R, isOutput=False)
    out_d = nc.declare_dram_parameter("out", [T, D], F32, isOutput=True)
    with tile.TileContext(nc) as tc:
        for _ in range(repeats):
            build_kernel(nc, tc, xT_d, wqkv_d, bqkv_d, rmsw_d, wo_d, out_d)
    split_multi_waits(nc)
    return nc


_NC_CACHE = None


def _get_nc():
    global _NC_CACHE
    if _NC_CACHE is None:
        _NC_CACHE = build_nc()
    return _NC_CACHE


def make_in_maps(x, w_q, b_q, w_k, b_k, w_v, b_v, rms_q_w, rms_k_w, w_o):
    xT = np.ascontiguousarray(x.reshape(T, D).T.astype(np.float32))
    # fold the 1/sqrt(HD) score scale into the q-side rms weight
    rq = (rms_q_w / np.sqrt(HD)).astype(np.float32)
    rmsw = np.concatenate([rq, rq, rms_k_w, rms_k_w]).astype(np.float32)
    rmsw = np.ascontiguousarray(rmsw[None, :])
    in_maps = []
    for c in range(NCORES):
        qs = slice(c * QKW, (c + 1) * QKW)
        vs = slice((c // 2) * VW, (c // 2 + 1) * VW)
        wqkv = np.ascontiguousarray(
            np.concatenate([w_q[:, qs], w_k[:, qs], w_v[:, vs]], axis=1)
        ).astype(np.float32)
        bqkv = np.ascontiguousarray(
            np.concatenate([b_q[qs], b_k[qs], b_v[vs]])[None, :]
        ).astype(np.float32)
        wo = np.ascontiguousarray(w_o[qs, :]).astype(np.float32)
        in_maps.append(
            {"xT": xT, "wqkv": wqkv, "bqkv": bqkv, "rmsw": rmsw, "wo": wo}
        )
    return in_maps


def kernel(x, w_q, b_q, w_k, b_k, w_v, b_v, rms_q_w, rms_k_w, w_o, b_o, **kw):
    x = np.asarray(x, np.float32)
    args = [np.asarray(a, np.float32) for a in
            (w_q, b_q, w_k, b_k, w_v, b_v, rms_q_w, rms_k_w, w_o)]
    in_maps = make_in_maps(x, *args)
    nc = _get_nc()
    res = run_bass_kernel_spmd(nc, in_maps, core_ids=list(range(NCORES)), **kw)
    acc = np.zeros((T, D), np.float64)
    for c in range(NCORES):
        acc += res.results[c]["out"].astype(np.float64)
    out = (acc + np.asarray(b_o, np.float64)[None, :]).astype(np.float32)
    return out.reshape(1, T, D)



# revision 10
# speedup vs baseline: 1.4794x; 1.4794x over previous
"""Multi-head "genetic" attention (windowed-causal, GQA) for Trainium2.

Self-contained: kernel(**inputs) takes full inputs, shards across 8
NeuronCores (2 query heads per core; value head h//4 per GQA), runs a
Bass/Tile kernel per core, and reduces the row-sharded output projection
partials on host.

Precision strategy: x / qkv weights / scores run in bf16 (the score path
is scale-insensitive), exp-weights and v run in fp16 (1 cyc/col on the
PE vs fp32r's 4 for <256-col outputs; fp16 keeps the tiny fitness
deviation around e~1.0 that bf16 would round away), out projection in
fp32r, per-core output partials in fp16 summed in f64 on host.
Measured end-to-end rel err ~2e-3 against the f32 reference.

Shapes (hardcoded): x (1, 2048, 1024), H=16 heads, head_dim 64, HV=4
value heads, window 512 (causal band of 513).
"""

import numpy as np

import bass_rust
import concourse.bass as bass
import concourse.tile as tile
from concourse import mybir
from concourse.bass_utils import run_bass_kernel_spmd
from concourse.masks import make_identity

F32 = mybir.dt.float32
F32R = mybir.dt.float32r
BF16 = mybir.dt.bfloat16
F16 = mybir.dt.float16
AF = mybir.ActivationFunctionType
ALU = mybir.AluOpType

T, D, H, HD, HV, WIN = 2048, 1024, 16, 64, 4, 512
NCORES = 8
HPC = H // NCORES          # 2 heads per core
P = 128
TT = T // P                # 16 t-tiles
KT = D // P                # 8 k-tiles over d_model
QKW = HPC * HD             # 128 q (or k) columns per core
VW = HD                    # 64 v columns per core
QKVW = 2 * QKW + VW        # 320 fused projection columns
EPS = 1.1920929e-07
NB = WIN // P + 1          # 5 band s-tiles max
MASK_FILL = -1.0e6         # exp(fill * fitness) == 0 for any fitness here

# ---------------------------------------------------------------------------
# This walrus build rejects >1 sem wait per instruction ("Too many sync wait
# commands"). Move extra waits onto same-engine NOPs inserted just before the
# offending instruction (engine queues are in-order, so blocking on the NOP
# is equivalent to blocking on the instruction itself).
_MAX_WAITS = 1


def split_multi_waits(nc, max_waits=_MAX_WAITS):
    for bb in nc.main_func.blocks:
        insts = bb.instructions
        i = 0
        while i < len(insts):
            inst = insts[i]
            si = inst.sync_info
            waits = list(si.on_wait or []) if si is not None else []
            if len(waits) > max_waits:
                si.on_wait = waits[-max_waits:]
                extra = waits[:-max_waits]
                nops = []
                for j in range(0, len(extra), max_waits):
                    n = nc.engines[inst.engine].nop(nofuse=True)
                    ni = n.ins
                    for bb2 in nc.main_func.blocks:
                        if ni in bb2.instructions:
                            bb2.instructions.remove(ni)
                            break
                    chunk = extra[j : j + max_waits]
                    if ni.sync_info is None:
                        ni.sync_info = bass_rust.SyncInfo(on_wait=chunk, on_update=[])
                    else:
                        ni.sync_info.on_wait = chunk
                    nops.append(ni)
                for k, ni in enumerate(nops):
                    insts.insert(i + k, ni)
                i += len(nops)
            i += 1
# ---------------------------------------------------------------------------


def _broadcast_row_ap(dram_ap, width):
    """DRAM AP replicating a (1, width) row across all 128 partitions."""
    return bass.AP(
        tensor=dram_ap.tensor,
        offset=dram_ap.offset,
        ap=[[0, P], [1, width]],
    )


def build_kernel(nc, tc, xT_d, wqkv_d, wo_d, out_d, bqkv_d, rmsw_d):
    from contextlib import ExitStack

    has_bias = bqkv_d is not None
    has_rmsw = rmsw_d is not None

    with ExitStack() as ctx:
        consts = ctx.enter_context(tc.tile_pool(name="consts", bufs=1))
        persist = ctx.enter_context(tc.tile_pool(name="persist", bufs=1))

        # ---- input DMAs first: big contiguous per-ko chunks -------------
        xT_sb = persist.tile([P, KT, T], BF16)
        for ko in range(KT):
            nc.sync.dma_start(
                xT_sb[:, ko, :], xT_d[ko * P : (ko + 1) * P, :]
            )
        wqkv_sb = persist.tile([P, KT, QKVW], BF16)
        for ko in range(KT):
            nc.sync.dma_start(
                wqkv_sb[:, ko, :], wqkv_d[ko * P : (ko + 1) * P, :]
            )
        wo_sb = persist.tile([P, D], F32R)
        nc.sync.dma_start(wo_sb, wo_d[:])
        if has_bias:
            bqkv_sb = consts.tile([1, QKVW], BF16)
            nc.sync.dma_start(bqkv_sb, bqkv_d[:])
        if has_rmsw:
            rmsw_b = consts.tile([P, 2 * QKW], F32)
            nc.gpsimd.dma_start(rmsw_b, _broadcast_row_ap(rmsw_d[:], 2 * QKW))

        # ---- constants ---------------------------------------------------
        ident_bf = consts.tile([P, P], BF16)
        make_identity(nc, ident_bf)
        ident_f = consts.tile([P, P], F32)
        make_identity(nc, ident_f)

        eps_t = consts.tile([P, 1], F32)
        nc.vector.memset(eps_t, EPS)
        eps_hd = consts.tile([P, 1], F32)
        nc.vector.memset(eps_hd, EPS * HD)
        ones_f = consts.tile([P, 1], F32)
        nc.vector.memset(ones_f, 1.0)
        if has_bias:
            ones1 = consts.tile([1, P], BF16)
            nc.vector.tensor_copy(ones1, ones_f[0:1, 0:1].to_broadcast((1, P)))

        # 127 - p, used for the partial-band diagonal t-tiles (s_lo == 0)
        causal_cnt = consts.tile([P, P], F32)
        nc.gpsimd.memset(causal_cnt, 1.0)
        nc.gpsimd.affine_select(
            out=causal_cnt, in_=causal_cnt, compare_op=ALU.is_ge, fill=0.0,
            base=0, pattern=[[-1, P]], channel_multiplier=1,
        )
        corr_lt = consts.tile([P, 1], F32)
        nc.vector.reduce_sum(corr_lt, causal_cnt, axis=mybir.AxisListType.X)
        nc.vector.tensor_scalar(corr_lt, corr_lt, -1.0, 128.0, ALU.mult, ALU.add)

        # denominator offset per t-tile: rs/T + Cvec ; in-strip masked slots
        # produce sigmoid(-1e6)=0, so their 0.5 contribution moves here.
        cvec = {}
        for tt in range(min(NB - 1, TT)):
            W = (tt + 1) * P
            c_base = 0.5 * (T - W) / T + 0.5
            cv = consts.tile([P, 1], F32, tag=f"cvec{tt}")
            nc.vector.tensor_scalar(cv, corr_lt, 0.5 / T, c_base, ALU.mult, ALU.add)
            cvec[tt] = cv
        C_FULL = 0.5 * (T - NB * P + (P - 1)) / T + 0.5

        fill_mask = nc.gpsimd.to_reg(MASK_FILL)
        fill_zero = nc.gpsimd.to_reg(0.0)

        ones2 = consts.tile([2, P], F32)
        nc.vector.memset(ones2, 1.0)

        qT = persist.tile([P, T], BF16)     # rows: head0 dims 0-63, head1 64-127
        kT = persist.tile([P, T], BF16)
        vN = persist.tile([P, TT, VW + 2], F16)  # v natural + ones cols (row sums)
        recip_all = persist.tile([P, HPC, TT], F32)
        nc.vector.tensor_copy(
            vN[:, :, VW : VW + 2],
            ones_f[:, :, None].to_broadcast((P, TT, 2)),
        )
        fs_all = persist.tile([P, HPC, TT], F32)

        # ---------------- Phase A: QKV projection + RMSNorm + transposes
        # Software-pipelined: the projection matmuls for tile tt+1 are
        # emitted before tile tt's normalization tail so the PE queue never
        # stalls on the vector/scalar chain.
        a_ctx = ExitStack()
        a_sb = a_ctx.enter_context(tc.tile_pool(name="a_sb", bufs=3))
        a_ps = a_ctx.enter_context(tc.tile_pool(name="a_ps", bufs=3, space="PSUM"))
        a_tr = a_ctx.enter_context(tc.tile_pool(name="a_tr", bufs=3, space="PSUM"))

        def emit_proj(tt):
            qkv_ps = a_ps.tile([P, QKVW], F32, tag="qkv")
            for ko in range(KT):
                nc.tensor.matmul(
                    qkv_ps,
                    lhsT=xT_sb[:, ko, tt * P : (tt + 1) * P],
                    rhs=wqkv_sb[:, ko, :],
                    start=(ko == 0),
                    stop=(ko == KT - 1 and not has_bias),
                )
            if has_bias:
                nc.tensor.matmul(
                    qkv_ps, lhsT=ones1, rhs=bqkv_sb, start=False, stop=True,
                )
            return qkv_ps

        def emit_norm(tt, qkv_ps):
            qk_ps = qkv_ps[:, : 2 * QKW].rearrange("p (c d) -> p c d", d=HD)
            sq = a_sb.tile([P, 2 * QKW], F32, tag="sq")
            nc.scalar.activation(sq, qkv_ps[:, : 2 * QKW], AF.Square)
            ssum = a_sb.tile([P, 4], F32, tag="ssum")
            nc.vector.reduce_sum(
                ssum, sq.rearrange("p (c d) -> p c d", d=HD),
                axis=mybir.AxisListType.X,
            )
            # q side folds the 1/sqrt(HD) score scale into the rms factor:
            # 1/(sqrt(HD)*sqrt(ssum/HD+eps)) == 1/sqrt(ssum + HD*eps)
            fac = a_sb.tile([P, 4], F32, tag="fac")
            nc.scalar.activation(fac[:, :2], ssum[:, :2], AF.Sqrt, bias=eps_hd)
            nc.scalar.activation(
                fac[:, 2:], ssum[:, 2:], AF.Sqrt, bias=eps_t, scale=1.0 / HD
            )
            rfac = a_sb.tile([P, 4], F32, tag="rfac")
            nc.vector.reciprocal(rfac, fac)
            qkn = a_sb.tile([P, 4, HD], BF16, tag="qkn")
            nc.vector.tensor_tensor(
                qkn, qk_ps, rfac[:, :, None].to_broadcast((P, 4, HD)), ALU.mult
            )
            if has_rmsw:
                nc.vector.tensor_tensor(
                    qkn, qkn,
                    rmsw_b.rearrange("p (c d) -> p c d", d=HD), ALU.mult,
                )
            for j, dst in ((0, qT), (1, kT)):
                ps = a_tr.tile([P, P], BF16, tag="tr")
                nc.tensor.transpose(
                    ps, qkn[:, 2 * j : 2 * j + 2, :].rearrange("p c d -> p (c d)"),
                    ident_bf,
                )
                if j == 0:
                    nc.vector.tensor_copy(dst[:, tt * P : (tt + 1) * P], ps)
                else:
                    nc.scalar.copy(dst[:, tt * P : (tt + 1) * P], ps)
            nc.vector.tensor_copy(vN[:, tt, :VW], qkv_ps[:, 2 * QKW :])

        prev = None
        for tt in range(TT):
            cur = emit_proj(tt)
            if prev is not None:
                emit_norm(*prev)
            prev = (tt, cur)
        emit_norm(*prev)
        a_ctx.close()

        # ---------------- Pass 1: banded scores (bf16), sigmoid stats
        strips = {}
        strip_pool = ctx.enter_context(tc.tile_pool(name="strips", bufs=1))
        with tc.tile_pool(name="p1_sb", bufs=3) as p1_sb, \
             tc.tile_pool(name="p1_ps", bufs=3, space="PSUM") as p1_ps:
            for tt in range(TT):
                s_lo = max(0, tt - (NB - 1))
                nst = tt - s_lo + 1
                W = nst * P
                for h in range(HPC):
                    ps = p1_ps.tile([P, NB * P], F32, tag="S")
                    for c0 in range(0, W, 512):
                        cw = min(512, W - c0)
                        nc.tensor.matmul(
                            ps[:, c0 : c0 + cw],
                            lhsT=qT[h * HD : (h + 1) * HD, tt * P : (tt + 1) * P],
                            rhs=kT[h * HD : (h + 1) * HD,
                                   s_lo * P + c0 : s_lo * P + c0 + cw],
                            start=True, stop=True,
                        )
                    strip = strip_pool.tile([P, W], BF16, tag=f"st{h}_{tt}")
                    strips[(h, tt)] = strip
                    nc.vector.tensor_copy(strip, ps[:, :W])
                    # band masking: keep c <= p on the diagonal tile,
                    # c >= p on the leading tile of full strips
                    nc.gpsimd.affine_select(
                        out=strip[:, W - P : W], in_=strip[:, W - P : W],
                        compare_op=ALU.is_ge, fill=fill_mask,
                        base=0, pattern=[[-1, P]], channel_multiplier=1,
                    )
                    if nst == NB:
                        nc.gpsimd.affine_select(
                            out=strip[:, :P], in_=strip[:, :P],
                            compare_op=ALU.is_ge, fill=fill_mask,
                            base=0, pattern=[[1, P]], channel_multiplier=-1,
                        )

                    sig = p1_sb.tile([P, NB * P], BF16, tag="sig")
                    rs = p1_sb.tile([P, 1], F32, tag="rs")
                    nc.scalar.activation(sig[:, :W], strip, AF.Sigmoid, accum_out=rs)
                    den = p1_sb.tile([P, 1], F32, tag="den")
                    if nst == NB:
                        nc.vector.tensor_scalar(den, rs, 1.0 / T, C_FULL,
                                                ALU.mult, ALU.add)
                    else:
                        nc.vector.tensor_scalar(den, rs, 1.0 / T, cvec[tt],
                                                ALU.mult, ALU.add)
                    nc.vector.reciprocal(recip_all[:, h, tt : tt + 1], den)

            # gene fitness scale per (head, t): recip(t) / sum_t recip(t).
            # Cross-partition sum via PE ones-reduction, then an on-chip
            # outer-product broadcast of the two per-head scalars.
            rsum = p1_sb.tile([P, HPC], F32, tag="rsum")
            nc.vector.reduce_sum(rsum, recip_all, axis=mybir.AxisListType.X)
            with tc.tile_pool(name="p1_sp", bufs=1, space="PSUM") as p1_sp:
                sinv_ps = p1_sp.tile([HPC, 1], F32, tag="sp")
                nc.tensor.matmul(sinv_ps, lhsT=rsum, rhs=ones_f,
                                 start=True, stop=True)
                sinv_r = p1_sb.tile([HPC, 1], F32, tag="sinvr")
                nc.vector.reciprocal(sinv_r, sinv_ps)
                # broadcast the two per-head scalars across partitions:
                # ones2.T @ diag(sinv_r) puts [s0, s1] on every partition
                diag2 = p1_sb.tile([HPC, HPC], F32, tag="diag2")
                nc.vector.tensor_copy(diag2, sinv_r.to_broadcast((HPC, HPC)))
                nc.gpsimd.affine_select(
                    out=diag2, in_=diag2, compare_op=ALU.is_equal, fill=fill_zero,
                    base=0, pattern=[[-1, HPC]], channel_multiplier=1,
                )
                srb_ps = p1_sp.tile([P, HPC], F32, tag="srbp")
                nc.tensor.matmul(srb_ps, lhsT=ones2, rhs=diag2,
                                 start=True, stop=True)
                srb = p1_sb.tile([P, HPC], F32, tag="srb")
                nc.vector.tensor_copy(srb, srb_ps)
            nc.vector.tensor_tensor(
                fs_all, recip_all,
                srb[:, :, None].to_broadcast((P, HPC, TT)), ALU.mult,
            )

        # ---------------- Pass 2: fitness-scaled strips, transpose, exp
        # (fp16), AV, output projection. 3-stage software pipeline keeps the
        # PE queue ahead of the scalar exp and vector tails.
        p2_sb = ctx.enter_context(tc.tile_pool(name="p2_sb", bufs=3))
        eT_pool = ctx.enter_context(tc.tile_pool(name="p2_eT", bufs=5))
        at_pool = ctx.enter_context(tc.tile_pool(name="p2_at", bufs=3))
        p2_wt = ctx.enter_context(tc.tile_pool(name="p2_wt", bufs=2, space="PSUM"))
        p2_av = ctx.enter_context(tc.tile_pool(name="p2_av", bufs=2, space="PSUM"))
        p2_tp = ctx.enter_context(tc.tile_pool(name="p2_tp", bufs=2, space="PSUM"))
        p2_o = ctx.enter_context(tc.tile_pool(name="p2_o", bufs=2, space="PSUM"))

        eTs = {}
        avs = {}
        attns = {}

        def stage1(tt):  # scale strips by fitness, transpose, exp -> fp16
            s_lo = max(0, tt - (NB - 1))
            nst = tt - s_lo + 1
            for h in range(HPC):
                strip = strips[(h, tt)]
                nc.vector.tensor_scalar(
                    strip, strip, fs_all[:, h, tt : tt + 1], None, ALU.mult
                )
                wt_ps = p2_wt.tile([P, NB, P], BF16, tag="wt")
                for st in range(nst):
                    nc.tensor.transpose(
                        wt_ps[:, st, :], strip[:, st * P : (st + 1) * P], ident_bf
                    )
                eT = eT_pool.tile([P, NB, P], F16, tag="eT")
                nc.scalar.activation(eT[:, :nst, :], wt_ps[:, :nst, :], AF.Exp)
                eTs[(h, tt)] = eT

        def stage2(tt):  # AV + softmax normalize
            s_lo = max(0, tt - (NB - 1))
            nst = tt - s_lo + 1
            attn = p2_sb.tile([P, QKW], F32, tag="attn")
            attns[tt] = attn
            for h in range(HPC):
                eT = eTs.pop((h, tt))
                av_ps = p2_av.tile([P, VW + 2], F32, tag="av")
                for st in range(nst):
                    nc.tensor.matmul(
                        av_ps, lhsT=eT[:, st, :], rhs=vN[:, s_lo + st, :],
                        start=(st == 0), stop=(st == nst - 1),
                    )
                erec = p2_sb.tile([P, 1], F32, tag="erec")
                nc.vector.reciprocal(erec, av_ps[:, VW : VW + 1])
                nc.vector.tensor_tensor(
                    attn[:, h * VW : (h + 1) * VW], av_ps[:, :VW],
                    erec.to_broadcast((P, VW)), ALU.mult,
                )

        def stage3(tt):  # transpose attn, output projection, store fp16
            attn = attns.pop(tt)
            atp = p2_tp.tile([P, P], F32, tag="atp")
            nc.tensor.transpose(atp, attn, ident_f)
            atT = at_pool.tile([P, P], F32R, tag="atT")
            nc.vector.tensor_copy(atT, atp)
            osb = p2_sb.tile([P, D], F16, tag="osb")
            for ci, c0 in enumerate(range(0, D, 512)):
                ops = p2_o.tile([P, 512], F32, tag="o")
                nc.tensor.matmul(
                    ops, lhsT=atT, rhs=wo_sb[:, c0 : c0 + 512],
                    start=True, stop=True,
                )
                nc.vector.tensor_copy(osb[:, c0 : c0 + 512], ops)
            nc.sync.dma_start(out_d[tt * P : (tt + 1) * P, :], osb)

        for tt in range(TT + 2):
            if tt < TT:
                stage1(tt)
            if 1 <= tt < TT + 1:
                stage2(tt - 1)
            if tt >= 2:
                stage3(tt - 2)


def build_nc(has_bias, has_rmsw):
    nc = bass.Bass()
    xT_d = nc.declare_dram_parameter("xT", [D, T], BF16, isOutput=False)
    wqkv_d = nc.declare_dram_parameter("wqkv", [D, QKVW], BF16, isOutput=False)
    wo_d = nc.declare_dram_parameter("wo", [QKW, D], F32R, isOutput=False)
    bqkv_d = (
        nc.declare_dram_parameter("bqkv", [1, QKVW], BF16, isOutput=False)
        if has_bias else None
    )
    rmsw_d = (
        nc.declare_dram_parameter("rmsw", [1, 2 * QKW], F32, isOutput=False)
        if has_rmsw else None
    )
    out_d = nc.declare_dram_parameter("out", [T, D], F16, isOutput=True)
    with tile.TileContext(nc) as tc:
        build_kernel(nc, tc, xT_d, wqkv_d, wo_d, out_d, bqkv_d, rmsw_d)
    split_multi_waits(nc)
    return nc


_NC_CACHE = {}
_LAST_FLAGS = (False, False)


def _get_nc(flags=None):
    global _NC_CACHE
    if flags is None:
        flags = _LAST_FLAGS
    if flags not in _NC_CACHE:
        _NC_CACHE[flags] = build_nc(*flags)
    return _NC_CACHE[flags]


def make_in_maps(x, w_q, b_q, w_k, b_k, w_v, b_v, rms_q_w, rms_k_w, w_o):
    global _LAST_FLAGS
    import ml_dtypes

    bf16 = ml_dtypes.bfloat16
    has_bias = bool(np.any(b_q) or np.any(b_k) or np.any(b_v))
    has_rmsw = not (
        np.all(rms_q_w == 1.0) and np.all(rms_k_w == 1.0)
    )
    _LAST_FLAGS = (has_bias, has_rmsw)

    xT = np.ascontiguousarray(x.reshape(T, D).T).astype(bf16)
    in_maps = []
    for c in range(NCORES):
        qs = slice(c * QKW, (c + 1) * QKW)
        vs = slice((c // 2) * VW, (c // 2 + 1) * VW)
        wqkv = np.ascontiguousarray(
            np.concatenate([w_q[:, qs], w_k[:, qs], w_v[:, vs]], axis=1)
        ).astype(bf16)
        wo = np.ascontiguousarray(w_o[qs, :]).astype(np.float32)
        m = {"xT": xT, "wqkv": wqkv, "wo": wo}
        if has_bias:
            m["bqkv"] = np.ascontiguousarray(
                np.concatenate([b_q[qs], b_k[qs], b_v[vs]])[None, :]
            ).astype(bf16)
        if has_rmsw:
            m["rmsw"] = np.ascontiguousarray(
                np.concatenate([rms_q_w, rms_q_w, rms_k_w, rms_k_w])[None, :]
            ).astype(np.float32)
        in_maps.append(m)
    return in_maps


def kernel(x, w_q, b_q, w_k, b_k, w_v, b_v, rms_q_w, rms_k_w, w_o, b_o, **kw):
    x = np.asarray(x, np.float32)
    args = [np.asarray(a, np.float32) for a in
            (w_q, b_q, w_k, b_k, w_v, b_v, rms_q_w, rms_k_w, w_o)]
    in_maps = make_in_maps(x, *args)
    nc = _get_nc()
    res = run_bass_kernel_spmd(nc, in_maps, core_ids=list(range(NCORES)), **kw)
    acc = np.zeros((T, D), np.float64)
    for c in range(NCORES):
        acc += res.results[c]["out"].astype(np.float64)
    out = (acc + np.asarray(b_o, np.float64)[None, :]).astype(np.float32)
    return out.reshape(1, T, D)


# revision 22
# speedup vs baseline: 1.5222x; 1.0290x over previous
"""Multi-head "genetic" attention (windowed-causal, GQA) for Trainium2.

Self-contained: kernel(**inputs) takes full inputs, shards across 8
NeuronCores (2 query heads per core; value head h//4 per GQA), runs a
Bass/Tile kernel per core, and reduces the row-sharded output projection
partials on host.

Precision strategy: x / qkv weights / scores run in bf16 (the score path
is scale-insensitive), exp-weights and v run in fp16 (1 cyc/col on the
PE vs fp32r's 4 for <256-col outputs; fp16 keeps the tiny fitness
deviation around e~1.0 that bf16 would round away), out projection in
fp32r, per-core output partials in fp16 summed in f64 on host.
Measured end-to-end rel err ~2e-3 against the f32 reference.

Shapes (hardcoded): x (1, 2048, 1024), H=16 heads, head_dim 64, HV=4
value heads, window 512 (causal band of 513).
"""

import numpy as np

import bass_rust
import concourse.bass as bass
import concourse.tile as tile
from concourse import mybir
from concourse.bass_utils import run_bass_kernel_spmd
from concourse.masks import make_identity

F32 = mybir.dt.float32
F32R = mybir.dt.float32r
BF16 = mybir.dt.bfloat16
F16 = mybir.dt.float16
AF = mybir.ActivationFunctionType
ALU = mybir.AluOpType

T, D, H, HD, HV, WIN = 2048, 1024, 16, 64, 4, 512
NCORES = 8
HPC = H // NCORES          # 2 heads per core
P = 128
TT = T // P                # 16 t-tiles
KT = D // P                # 8 k-tiles over d_model
QKW = HPC * HD             # 128 q (or k) columns per core
VW = HD                    # 64 v columns per core
QKVW = 2 * QKW + VW        # 320 fused projection columns
EPS = 1.1920929e-07
NB = WIN // P + 1          # 5 band s-tiles max
MASK_FILL = -1.0e6         # exp(fill * fitness) == 0 for any fitness here

# ---------------------------------------------------------------------------
# This walrus build rejects >1 sem wait per instruction ("Too many sync wait
# commands"). Move extra waits onto same-engine NOPs inserted just before the
# offending instruction (engine queues are in-order, so blocking on the NOP
# is equivalent to blocking on the instruction itself).
_MAX_WAITS = 1


def split_multi_waits(nc, max_waits=_MAX_WAITS):
    for bb in nc.main_func.blocks:
        insts = bb.instructions
        i = 0
        while i < len(insts):
            inst = insts[i]
            si = inst.sync_info
            waits = list(si.on_wait or []) if si is not None else []
            if len(waits) > max_waits:
                si.on_wait = waits[-max_waits:]
                extra = waits[:-max_waits]
                nops = []
                for j in range(0, len(extra), max_waits):
                    n = nc.engines[inst.engine].nop(nofuse=True)
                    ni = n.ins
                    for bb2 in nc.main_func.blocks:
                        if ni in bb2.instructions:
                            bb2.instructions.remove(ni)
                            break
                    chunk = extra[j : j + max_waits]
                    if ni.sync_info is None:
                        ni.sync_info = bass_rust.SyncInfo(on_wait=chunk, on_update=[])
                    else:
                        ni.sync_info.on_wait = chunk
                    nops.append(ni)
                for k, ni in enumerate(nops):
                    insts.insert(i + k, ni)
                i += len(nops)
            i += 1
# ---------------------------------------------------------------------------


def _broadcast_row_ap(dram_ap, width):
    """DRAM AP replicating a (1, width) row across all 128 partitions."""
    return bass.AP(
        tensor=dram_ap.tensor,
        offset=dram_ap.offset,
        ap=[[0, P], [1, width]],
    )


def build_kernel(nc, tc, xT_d, wqkv_d, wo_d, out_d, bqkv_d, rmsw_d):
    from contextlib import ExitStack

    has_bias = bqkv_d is not None
    has_rmsw = rmsw_d is not None

    with ExitStack() as ctx:
        consts = ctx.enter_context(tc.tile_pool(name="consts", bufs=1))
        persist = ctx.enter_context(tc.tile_pool(name="persist", bufs=1))

        # ---- input DMAs first: big contiguous per-ko chunks. Descriptor
        # generation is serial per HWDGE ring, so split the weight loads
        # onto the Activation ring (scalar) while x rides the SP ring.
        xT_sb = persist.tile([P, KT, T], BF16)
        wqkv_sb = persist.tile([P, KT, QKVW], BF16)
        nc.sync.dma_start(xT_sb[:, 0, :], xT_d[0:P, :])
        for ko in range(KT):
            nc.scalar.dma_start(
                wqkv_sb[:, ko, :], wqkv_d[ko * P : (ko + 1) * P, :]
            )
        for ko in range(1, KT):
            nc.sync.dma_start(
                xT_sb[:, ko, :], xT_d[ko * P : (ko + 1) * P, :]
            )
        wo_sb = persist.tile([P, D], F32R)
        nc.scalar.dma_start(wo_sb, wo_d[:])
        if has_bias:
            bqkv_sb = consts.tile([1, QKVW], BF16)
            nc.scalar.dma_start(bqkv_sb, bqkv_d[:])
        if has_rmsw:
            rmsw_b = consts.tile([P, 2 * QKW], F32)
            nc.gpsimd.dma_start(rmsw_b, _broadcast_row_ap(rmsw_d[:], 2 * QKW))

        # ---- constants ---------------------------------------------------
        ident_bf = consts.tile([P, P], BF16)
        make_identity(nc, ident_bf)
        ident_f = consts.tile([P, P], F32)
        make_identity(nc, ident_f)

        eps_t = consts.tile([P, 1], F32)
        nc.vector.memset(eps_t, EPS)
        ones_f = consts.tile([P, 1], F32)
        nc.vector.memset(ones_f, 1.0)
        if has_bias:
            ones1 = consts.tile([1, P], BF16)
            nc.vector.tensor_copy(ones1, ones_f[0:1, 0:1].to_broadcast((1, P)))

        # 127 - p, used for the partial-band diagonal t-tiles (s_lo == 0)
        causal_cnt = consts.tile([P, P], F32)
        nc.gpsimd.memset(causal_cnt, 1.0)
        nc.gpsimd.affine_select(
            out=causal_cnt, in_=causal_cnt, compare_op=ALU.is_ge, fill=0.0,
            base=0, pattern=[[-1, P]], channel_multiplier=1,
        )
        corr_lt = consts.tile([P, 1], F32)
        nc.vector.reduce_sum(corr_lt, causal_cnt, axis=mybir.AxisListType.X)
        nc.vector.tensor_scalar(corr_lt, corr_lt, -1.0, 128.0, ALU.mult, ALU.add)

        # denominator offset per t-tile: rs/T + Cvec ; in-strip masked slots
        # produce sigmoid(-1e6)=0, so their 0.5 contribution moves here.
        cvec = {}
        for tt in range(min(NB - 1, TT)):
            W = (tt + 1) * P
            c_base = 0.5 * (T - W) / T + 0.5
            cv = consts.tile([P, 1], F32, tag=f"cvec{tt}")
            nc.vector.tensor_scalar(cv, corr_lt, 0.5 / T, c_base, ALU.mult, ALU.add)
            cvec[tt] = cv
        C_FULL = 0.5 * (T - NB * P + (P - 1)) / T + 0.5

        fill_mask = nc.gpsimd.to_reg(MASK_FILL)
        fill_zero = nc.gpsimd.to_reg(0.0)

        # 1/sqrt(HD) score scaling is folded into the fitness broadcast (via
        # this 0.125 instead of 1.0) and into the sigmoid's free scale param,
        # so the q/k normalization needs no extra factor anywhere.
        ones2 = consts.tile([2, P], F32)
        nc.vector.memset(ones2, 1.0 / np.sqrt(HD))

        qT = persist.tile([P, T], BF16)     # rows: head0 dims 0-63, head1 64-127
        kT = persist.tile([P, T], BF16)
        vN = persist.tile([P, TT, VW + 2], F16)  # v natural + ones cols (row sums)
        recip_all = persist.tile([P, TT, HPC], F32)
        nc.vector.tensor_copy(
            vN[:, :, VW : VW + 2],
            ones_f[:, :, None].to_broadcast((P, TT, 2)),
        )
        fs_all = persist.tile([P, TT, HPC], F32)

        # ---------------- Phase A: QKV projection + RMSNorm + transposes
        # Software-pipelined: the projection matmuls for tile tt+1 are
        # emitted before tile tt's normalization tail so the PE queue never
        # stalls on the vector/scalar chain.
        a_ctx = ExitStack()
        a_sb = a_ctx.enter_context(tc.tile_pool(name="a_sb", bufs=3))
        a_ps = a_ctx.enter_context(tc.tile_pool(name="a_ps", bufs=3, space="PSUM"))
        a_tr = a_ctx.enter_context(tc.tile_pool(name="a_tr", bufs=3, space="PSUM"))

        def emit_proj(tt):
            qkv_ps = a_ps.tile([P, QKVW], F32, tag="qkv")
            for ko in range(KT):
                nc.tensor.matmul(
                    qkv_ps,
                    lhsT=xT_sb[:, ko, tt * P : (tt + 1) * P],
                    rhs=wqkv_sb[:, ko, :],
                    start=(ko == 0),
                    stop=(ko == KT - 1 and not has_bias),
                )
            if has_bias:
                nc.tensor.matmul(
                    qkv_ps, lhsT=ones1, rhs=bqkv_sb, start=False, stop=True,
                )
            return qkv_ps

        def emit_norm(tt, qkv_ps):
            qk_ps = qkv_ps[:, : 2 * QKW].rearrange("p (c d) -> p c d", d=HD)
            sq = a_sb.tile([P, 2 * QKW], F32, tag="sq")
            nc.scalar.activation(sq, qkv_ps[:, : 2 * QKW], AF.Square)
            ssum = a_sb.tile([P, 4], F32, tag="ssum")
            nc.vector.reduce_sum(
                ssum, sq.rearrange("p (c d) -> p c d", d=HD),
                axis=mybir.AxisListType.X,
            )
            fac = a_sb.tile([P, 4], F32, tag="fac")
            nc.scalar.activation(fac, ssum, AF.Sqrt, bias=eps_t, scale=1.0 / HD)
            rfac = a_sb.tile([P, 4], F32, tag="rfac")
            nc.vector.reciprocal(rfac, fac)
            qkn = a_sb.tile([P, 4, HD], BF16, tag="qkn")
            nc.vector.tensor_tensor(
                qkn, qk_ps, rfac[:, :, None].to_broadcast((P, 4, HD)), ALU.mult
            )
            if has_rmsw:
                nc.vector.tensor_tensor(
                    qkn, qkn,
                    rmsw_b.rearrange("p (c d) -> p c d", d=HD), ALU.mult,
                )
            for j, dst in ((0, qT), (1, kT)):
                ps = a_tr.tile([P, P], BF16, tag="tr")
                nc.tensor.transpose(
                    ps, qkn[:, 2 * j : 2 * j + 2, :].rearrange("p c d -> p (c d)"),
                    ident_bf,
                )
                nc.vector.tensor_copy(dst[:, tt * P : (tt + 1) * P], ps)
            nc.vector.tensor_copy(vN[:, tt, :VW], qkv_ps[:, 2 * QKW :])

        prev = None
        for tt in range(TT):
            cur = emit_proj(tt)
            if prev is not None:
                emit_norm(*prev)
            prev = (tt, cur)
        emit_norm(*prev)
        a_ctx.close()

        # ---------------- Pass 1: banded scores (bf16), sigmoid stats
        strips = {}
        strip_pool = ctx.enter_context(tc.tile_pool(name="strips", bufs=1))
        with tc.tile_pool(name="p1_sb", bufs=3) as p1_sb, \
             tc.tile_pool(name="p1_ps", bufs=3, space="PSUM") as p1_ps:
            for tt in range(TT):
                s_lo = max(0, tt - (NB - 1))
                nst = tt - s_lo + 1
                W = nst * P
                rs2 = p1_sb.tile([P, HPC], F32, tag="rs")
                for h in range(HPC):
                    ps = p1_ps.tile([P, NB * P], F32, tag="S")
                    for c0 in range(0, W, 512):
                        cw = min(512, W - c0)
                        nc.tensor.matmul(
                            ps[:, c0 : c0 + cw],
                            lhsT=qT[h * HD : (h + 1) * HD, tt * P : (tt + 1) * P],
                            rhs=kT[h * HD : (h + 1) * HD,
                                   s_lo * P + c0 : s_lo * P + c0 + cw],
                            start=True, stop=True,
                        )
                    strip = strip_pool.tile([P, W], BF16, tag=f"st{h}_{tt}")
                    strips[(h, tt)] = strip
                    nc.vector.tensor_copy(strip, ps[:, :W])
                    # band masking: keep c <= p on the diagonal tile,
                    # c >= p on the leading tile of full strips
                    nc.gpsimd.affine_select(
                        out=strip[:, W - P : W], in_=strip[:, W - P : W],
                        compare_op=ALU.is_ge, fill=fill_mask,
                        base=0, pattern=[[-1, P]], channel_multiplier=1,
                    )
                    if nst == NB:
                        nc.gpsimd.affine_select(
                            out=strip[:, :P], in_=strip[:, :P],
                            compare_op=ALU.is_ge, fill=fill_mask,
                            base=0, pattern=[[1, P]], channel_multiplier=-1,
                        )

                    sig = p1_sb.tile([P, NB * P], BF16, tag="sig")
                    nc.scalar.activation(
                        sig[:, :W], strip, AF.Sigmoid,
                        scale=1.0 / np.sqrt(HD), accum_out=rs2[:, h : h + 1],
                    )
                den2 = p1_sb.tile([P, HPC], F32, tag="den")
                cv = C_FULL if nst == NB else cvec[tt]
                nc.vector.tensor_scalar(den2, rs2, 1.0 / T, cv,
                                        ALU.mult, ALU.add)
                nc.vector.reciprocal(recip_all[:, tt, :], den2)

            # gene fitness scale per (head, t): recip(t) / sum_t recip(t).
            # Cross-partition sum via PE ones-reduction, then an on-chip
            # outer-product broadcast of the two per-head scalars.
            rsum = p1_sb.tile([P, HPC], F32, tag="rsum")
            for h in range(HPC):
                nc.vector.reduce_sum(
                    rsum[:, h : h + 1],
                    recip_all[:, :, h : h + 1].rearrange("p t o -> p (t o)"),
                    axis=mybir.AxisListType.X,
                )
            with tc.tile_pool(name="p1_sp", bufs=1, space="PSUM") as p1_sp:
                sinv_ps = p1_sp.tile([HPC, 1], F32, tag="sp")
                nc.tensor.matmul(sinv_ps, lhsT=rsum, rhs=ones_f,
                                 start=True, stop=True)
                sinv_r = p1_sb.tile([HPC, 1], F32, tag="sinvr")
                nc.vector.reciprocal(sinv_r, sinv_ps)
                # broadcast the two per-head scalars across partitions:
                # ones2.T @ diag(sinv_r) puts [s0, s1] on every partition
                diag2 = p1_sb.tile([HPC, HPC], F32, tag="diag2")
                nc.vector.tensor_copy(diag2, sinv_r.to_broadcast((HPC, HPC)))
                nc.gpsimd.affine_select(
                    out=diag2, in_=diag2, compare_op=ALU.is_equal, fill=fill_zero,
                    base=0, pattern=[[-1, HPC]], channel_multiplier=1,
                )
                srb_ps = p1_sp.tile([P, HPC], F32, tag="srbp")
                nc.tensor.matmul(srb_ps, lhsT=ones2, rhs=diag2,
                                 start=True, stop=True)
                srb = p1_sb.tile([P, HPC], F32, tag="srb")
                nc.vector.tensor_copy(srb, srb_ps)
            nc.vector.tensor_tensor(
                fs_all, recip_all,
                srb[:, None, :].to_broadcast((P, TT, HPC)), ALU.mult,
            )

        # ---------------- Pass 2: fitness-scaled strips, transpose, exp
        # (fp16), AV, output projection. 3-stage software pipeline keeps the
        # PE queue ahead of the scalar exp and vector tails.
        p2_sb = ctx.enter_context(tc.tile_pool(name="p2_sb", bufs=3))
        eT_pool = ctx.enter_context(tc.tile_pool(name="p2_eT", bufs=5))
        at_pool = ctx.enter_context(tc.tile_pool(name="p2_at", bufs=3))
        p2_wt = ctx.enter_context(tc.tile_pool(name="p2_wt", bufs=2, space="PSUM"))
        p2_av = ctx.enter_context(tc.tile_pool(name="p2_av", bufs=2, space="PSUM"))
        p2_tp = ctx.enter_context(tc.tile_pool(name="p2_tp", bufs=2, space="PSUM"))
        p2_o = ctx.enter_context(tc.tile_pool(name="p2_o", bufs=2, space="PSUM"))

        eTs = {}
        avs = {}
        attns = {}

        def stage1(tt):  # scale strips by fitness, transpose, exp -> fp16
            s_lo = max(0, tt - (NB - 1))
            nst = tt - s_lo + 1
            for h in range(HPC):
                strip = strips[(h, tt)]
                nc.vector.tensor_scalar(
                    strip, strip, fs_all[:, tt, h : h + 1], None, ALU.mult
                )
                wt_ps = p2_wt.tile([P, NB, P], BF16, tag="wt")
                for st in range(nst):
                    nc.tensor.transpose(
                        wt_ps[:, st, :], strip[:, st * P : (st + 1) * P], ident_bf
                    )
                eT = eT_pool.tile([P, NB, P], F16, tag="eT")
                nc.scalar.activation(eT[:, :nst, :], wt_ps[:, :nst, :], AF.Exp)
                eTs[(h, tt)] = eT

        def stage2(tt):  # AV + softmax normalize
            s_lo = max(0, tt - (NB - 1))
            nst = tt - s_lo + 1
            attn = p2_sb.tile([P, QKW], F32, tag="attn")
            attns[tt] = attn
            for h in range(HPC):
                eT = eTs.pop((h, tt))
                av_ps = p2_av.tile([P, VW + 2], F32, tag="av")
                for st in range(nst):
                    nc.tensor.matmul(
                        av_ps, lhsT=eT[:, st, :], rhs=vN[:, s_lo + st, :],
                        start=(st == 0), stop=(st == nst - 1),
                    )
                erec = p2_sb.tile([P, 1], F32, tag="erec")
                nc.vector.reciprocal(erec, av_ps[:, VW : VW + 1])
                nc.vector.tensor_tensor(
                    attn[:, h * VW : (h + 1) * VW], av_ps[:, :VW],
                    erec.to_broadcast((P, VW)), ALU.mult,
                )

        def stage3(tt):  # transpose attn, output projection, store fp16
            attn = attns.pop(tt)
            atp = p2_tp.tile([P, P], F32, tag="atp")
            nc.tensor.transpose(atp, attn, ident_f)
            atT = at_pool.tile([P, P], F32R, tag="atT")
            nc.vector.tensor_copy(atT, atp)
            osb = p2_sb.tile([P, D], F16, tag="osb")
            for ci, c0 in enumerate(range(0, D, 512)):
                ops = p2_o.tile([P, 512], F32, tag="o")
                nc.tensor.matmul(
                    ops, lhsT=atT, rhs=wo_sb[:, c0 : c0 + 512],
                    start=True, stop=True,
                )
                if ci == 0:
                    nc.vector.tensor_copy(osb[:, c0 : c0 + 512], ops)
                else:
                    nc.scalar.copy(osb[:, c0 : c0 + 512], ops)
            nc.sync.dma_start(out_d[tt * P : (tt + 1) * P, :], osb)

        for tt in range(TT + 2):
            if tt < TT:
                stage1(tt)
            if 1 <= tt < TT + 1:
                stage2(tt - 1)
            if tt >= 2:
                stage3(tt - 2)


def build_nc(has_bias, has_rmsw):
    nc = bass.Bass()
    xT_d = nc.declare_dram_parameter("xT", [D, T], BF16, isOutput=False)
    wqkv_d = nc.declare_dram_parameter("wqkv", [D, QKVW], BF16, isOutput=False)
    wo_d = nc.declare_dram_parameter("wo", [QKW, D], F32R, isOutput=False)
    bqkv_d = (
        nc.declare_dram_parameter("bqkv", [1, QKVW], BF16, isOutput=False)
        if has_bias else None
    )
    rmsw_d = (
        nc.declare_dram_parameter("rmsw", [1, 2 * QKW], F32, isOutput=False)
        if has_rmsw else None
    )
    out_d = nc.declare_dram_parameter("out", [T, D], F16, isOutput=True)
    with tile.TileContext(nc) as tc:
        build_kernel(nc, tc, xT_d, wqkv_d, wo_d, out_d, bqkv_d, rmsw_d)
    split_multi_waits(nc)
    return nc


_NC_CACHE = {}
_LAST_FLAGS = (False, False)


def _get_nc(flags=None):
    global _NC_CACHE
    if flags is None:
        flags = _LAST_FLAGS
    if flags not in _NC_CACHE:
        _NC_CACHE[flags] = build_nc(*flags)
    return _NC_CACHE[flags]


def make_in_maps(x, w_q, b_q, w_k, b_k, w_v, b_v, rms_q_w, rms_k_w, w_o):
    global _LAST_FLAGS
    import ml_dtypes

    bf16 = ml_dtypes.bfloat16
    has_bias = bool(np.any(b_q) or np.any(b_k) or np.any(b_v))
    has_rmsw = not (
        np.all(rms_q_w == 1.0) and np.all(rms_k_w == 1.0)
    )
    _LAST_FLAGS = (has_bias, has_rmsw)

    xT = np.ascontiguousarray(x.reshape(T, D).T).astype(bf16)
    in_maps = []
    for c in range(NCORES):
        qs = slice(c * QKW, (c + 1) * QKW)
        vs = slice((c // 2) * VW, (c // 2 + 1) * VW)
        wqkv = np.ascontiguousarray(
            np.concatenate([w_q[:, qs], w_k[:, qs], w_v[:, vs]], axis=1)
        ).astype(bf16)
        wo = np.ascontiguousarray(w_o[qs, :]).astype(np.float32)
        m = {"xT": xT, "wqkv": wqkv, "wo": wo}
        if has_bias:
            m["bqkv"] = np.ascontiguousarray(
                np.concatenate([b_q[qs], b_k[qs], b_v[vs]])[None, :]
            ).astype(bf16)
        if has_rmsw:
            m["rmsw"] = np.ascontiguousarray(
                np.concatenate([rms_q_w, rms_q_w, rms_k_w, rms_k_w])[None, :]
            ).astype(np.float32)
        in_maps.append(m)
    return in_maps


def kernel(x, w_q, b_q, w_k, b_k, w_v, b_v, rms_q_w, rms_k_w, w_o, b_o, **kw):
    x = np.asarray(x, np.float32)
    args = [np.asarray(a, np.float32) for a in
            (w_q, b_q, w_k, b_k, w_v, b_v, rms_q_w, rms_k_w, w_o)]
    in_maps = make_in_maps(x, *args)
    nc = _get_nc()
    res = run_bass_kernel_spmd(nc, in_maps, core_ids=list(range(NCORES)), **kw)
    acc = np.zeros((T, D), np.float64)
    for c in range(NCORES):
        acc += res.results[c]["out"].astype(np.float64)
    out = (acc + np.asarray(b_o, np.float64)[None, :]).astype(np.float32)
    return out.reshape(1, T, D)
